# revision 20
# baseline (speedup 1.0000x reference)
"""Trainium2 Bass kernel for nn_AssociationLayer (sparse-attention transformer block).

Sharding: pure data-parallel over batch. B=16 samples across 8 cores, 2 samples
per core, no collectives. Host pre-transposes K and folds LN gains / q-scale
into the weight matrices; the device computes LN -> qkv -> masked attention ->
proj -> residual -> LN -> MLP -> residual per sample.

Attention math (validated vs reference): with nrc = n1*n2,
  rows i <  nrc: softmax over keys j < nrc of (q_i.k_j/sqrt(D) + K[i,j]) @ v
  rows i >= nrc: uniform attention = mean over ALL keys of v
Scores are computed transposed (S^T[j,i], keys on partitions) so the key mask
and softmax shift ride the ACT exp bias, and exp(S^T) feeds the AV matmul as
lhsT with no transposes. Row sums come from a ones-column in v_aug; 1/s is
exp(-ln(s)) on ACT.
"""

import numpy as np

B, N, C = 16, 1024, 256
H, D = 4, 64
NCORES = 8
SPC = 2  # samples per core
P = 128
NT = N // P  # 8 token tiles
ICW = 512  # query-chunk width
NEG = -1.0e10
SHIFT = -12.0  # exp stability shift
EPS = 1e-5


def _build(R_max, C_max, parts="all"):
    import concourse.bass as bass
    import concourse.mybir as mybir
    import concourse.tile as tile
    from concourse import bacc

    f32 = mybir.dt.float32
    bf16 = mybir.dt.bfloat16
    Alu = mybir.AluOpType
    Act = mybir.ActivationFunctionType

    # Force Exp/Ln to resolve to the combined natural_log_exp set so the
    # greedy table-load pass doesn't ping-pong between exp_and_others and
    # natural_log (each reload costs ~2.7us on ACT). Indices are preserved.
    import concourse.hw_specs as hw_specs
    if not getattr(bacc, "_act_tables_patched", False):
        _orig_get_tables = hw_specs.get_activation_tables

        def _patched_tables(arch):
            tabs = dict(_orig_get_tables(arch))
            for nm in list(tabs.keys()):
                if nm != "natural_log_exp_and_others":
                    tabs[nm] = set(tabs[nm]) - {Act.Exp, Act.Ln}
            return tabs

        bacc.get_activation_tables = _patched_tables
        bacc._act_tables_patched = True

    nc = bacc.Bacc()

    # ---- DRAM parameters (order = in_maps keys) ----
    x_ext = nc.declare_dram_parameter("x", [SPC, N, C], f32, isOutput=False)
    kt_ext = nc.declare_dram_parameter("kt", [SPC, N, N], bf16, isOutput=False)
    nrcf_ext = nc.declare_dram_parameter("nrcf", [SPC, 1], f32, isOutput=False)
    wqk_ext = nc.declare_dram_parameter("wqk_t", [C, 2 * C], bf16, isOutput=False)
    bqk_ext = nc.declare_dram_parameter("bqk_col", [P, 4], f32, isOutput=False)
    wv_ext = nc.declare_dram_parameter("wv_t", [C, C], bf16, isOutput=False)
    bv_ext = nc.declare_dram_parameter("bv_col", [P, 2], f32, isOutput=False)
    bvrow_ext = nc.declare_dram_parameter("bv_row", [1, C], f32, isOutput=False)
    proj_ext = nc.declare_dram_parameter("proj_t", [C, C], bf16, isOutput=False)
    pbrow_ext = nc.declare_dram_parameter("pb_row", [1, C], bf16, isOutput=False)
    fc1_ext = nc.declare_dram_parameter("fc1_t", [C, 4 * C], bf16, isOutput=False)
    bf1_ext = nc.declare_dram_parameter("bf1_col", [P, 8], f32, isOutput=False)
    fc2_ext = nc.declare_dram_parameter("fc2_t", [4 * C, C], bf16, isOutput=False)
    f2b_ext = nc.declare_dram_parameter("f2b_row", [1, C], bf16, isOutput=False)
    sel_ext = nc.declare_dram_parameter("sel4", [P, C], f32, isOutput=False)
    id_ext = nc.declare_dram_parameter("ident", [P, P], bf16, isOutput=False)
    iota_ext = nc.declare_dram_parameter("iota_pt", [P, NT], f32, isOutput=False)
    meta_ext = nc.declare_dram_parameter("rflags", [1, 2 * SPC], mybir.dt.int32, isOutput=False)
    out_ext = nc.declare_dram_parameter("out", [SPC, N, C], f32, isOutput=True)
    hn_dram = nc.dram_tensor("hn_stage", [SPC, 2, N, C], bf16)  # [s, which_ln, tokens, chan]

    with tile.TileContext(nc) as tc:
        with (
            tc.tile_pool(name="singles", bufs=1) as singles,
            tc.tile_pool(name="big", bufs=2) as big,
            tc.tile_pool(name="big1", bufs=1) as big1,
            tc.tile_pool(name="pt2", bufs=2) as pt2,
            tc.tile_pool(name="hnp", bufs=8) as hnp,
            tc.tile_pool(name="epi", bufs=2) as epi,
            tc.tile_pool(name="work", bufs=3) as work,
            tc.tile_pool(name="stats", bufs=4) as stats,
            tc.tile_pool(name="outp", bufs=3) as outp,
            tc.tile_pool(name="psw", bufs=4, space="PSUM") as psw,
            tc.tile_pool(name="psacc", bufs=1, space="PSUM") as psacc,
        ):
            # ---- constants / weights resident in SBUF ----
            wqk_sb = singles.tile([P, 2, 2 * C], bf16, tag="wqk")
            nc.sync.dma_start(out=wqk_sb[:], in_=wqk_ext.rearrange("(c2 p) r -> p c2 r", p=P))
            wv_sb = singles.tile([P, 2, C], bf16, tag="wv")
            nc.sync.dma_start(out=wv_sb[:], in_=wv_ext.rearrange("(c2 p) r -> p c2 r", p=P))
            proj_sb = singles.tile([P, 2, C], bf16, tag="proj")
            nc.sync.dma_start(out=proj_sb[:], in_=proj_ext.rearrange("(c2 p) r -> p c2 r", p=P))
            fc1_sb = singles.tile([P, 2, 4 * C], bf16, tag="fc1")
            nc.sync.dma_start(out=fc1_sb[:], in_=fc1_ext.rearrange("(c2 p) r -> p c2 r", p=P))
            fc2_sb = singles.tile([P, 8, C], bf16, tag="fc2")
            nc.sync.dma_start(out=fc2_sb[:], in_=fc2_ext.rearrange("(r p) c -> p r c", p=P))
            bqk_sb = singles.tile([P, 4], f32, tag="bqk")
            nc.sync.dma_start(out=bqk_sb[:], in_=bqk_ext[:])
            bv_sb = singles.tile([P, 2], f32, tag="bv")
            nc.sync.dma_start(out=bv_sb[:], in_=bv_ext[:])
            bvrow_sb = singles.tile([1, C], f32, tag="bvrow")
            nc.sync.dma_start(out=bvrow_sb[:], in_=bvrow_ext[:])
            pbrow_sb = singles.tile([1, C], bf16, tag="pbrow")
            nc.sync.dma_start(out=pbrow_sb[:], in_=pbrow_ext[:])
            bf1_sb = singles.tile([P, 8], f32, tag="bf1")
            nc.sync.dma_start(out=bf1_sb[:], in_=bf1_ext[:])
            f2b_sb = singles.tile([1, C], bf16, tag="f2b")
            nc.sync.dma_start(out=f2b_sb[:], in_=f2b_ext[:])
            sel_sb = singles.tile([P, C], f32, tag="sel")
            nc.sync.dma_start(out=sel_sb[:], in_=sel_ext[:])
            id_sb = singles.tile([P, P], bf16, tag="ident")
            nc.sync.dma_start(out=id_sb[:], in_=id_ext[:])
            iota_sb = singles.tile([P, NT], f32, tag="iota")
            nc.sync.dma_start(out=iota_sb[:], in_=iota_ext[:])
            meta_sb = singles.tile([1, 2 * SPC], mybir.dt.int32, tag="meta")
            nc.sync.dma_start(out=meta_sb[:], in_=meta_ext[:])
            eps_sb = singles.tile([P, 1], f32, tag="eps")
            nc.gpsimd.memset(eps_sb[:], EPS)
            ones1_sb = singles.tile([1, P], f32, tag="ones1")
            nc.gpsimd.memset(ones1_sb[:], 1.0)
            ones1_bf = singles.tile([1, P], bf16, tag="ones1bf")
            nc.gpsimd.memset(ones1_bf[:], 1.0)
            ucol_sb = singles.tile([P, 1], bf16, tag="ucol")
            nc.gpsimd.memset(ucol_sb[:], 1.0 / N)
            stage_sb = singles.tile([P, ICW], f32, tag="stage")
            nc.gpsimd.memset(stage_sb[:], 1.0)
            zcol_sb = singles.tile([1, D + 1], bf16, tag="zcol")
            nc.gpsimd.memset(zcol_sb[:], 0.0)
            zrow_sb = singles.tile([1, ICW], bf16, tag="zrow")
            nc.gpsimd.memset(zrow_sb[:], 0.0)

            # per-sample persistent tiles
            x_sb = [singles.tile([P, NT, C], f32, tag=f"x{s}", name=f"x{s}") for s in range(SPC)]
            mval = [singles.tile([P, NT], f32, tag=f"mval{s}", name=f"mval{s}") for s in range(SPC)]
            minv = [singles.tile([P, NT], f32, tag=f"minv{s}", name=f"minv{s}") for s in range(SPC)]
            kb = [singles.tile([P, NT], f32, tag=f"kb{s}", name=f"kb{s}") for s in range(SPC)]
            ub_sb = [singles.tile([P, C], f32, tag=f"ub{s}", name=f"ub{s}") for s in range(SPC)]

            def layernorm_all(src3, s, which, hT):
                """LN over free dim C of all NT tiles of src3 -> hT [P, 2, N]
                (transposed, hT[c, i]) via a DRAM round-trip: 8 row DMAs out,
                2 large xbar transposes back. rstd batched in one Ln+Exp."""
                mean8 = stats.tile([P, NT], f32, tag="mean8")
                var8 = stats.tile([P, NT], f32, tag="var8")
                for t in range(NT):
                    st6 = stats.tile([P, 6], f32, tag="st6")
                    nc.vector.bn_stats(out=st6[:], in_=src3[:, t, :])
                    mv2 = stats.tile([P, 2], f32, tag="mv2")
                    nc.vector.bn_aggr(out=mv2[:], in_=st6[:])
                    nc.vector.tensor_copy(out=mean8[:, t:t + 1], in_=mv2[:, 0:1])
                    nc.vector.tensor_copy(out=var8[:, t:t + 1], in_=mv2[:, 1:2])
                lnv8 = stats.tile([P, NT], f32, tag="lnv8")
                nc.scalar.activation(out=lnv8[:], in_=var8[:], func=Act.Ln,
                                     bias=eps_sb[:], scale=1.0)
                rstd8 = stats.tile([P, NT], f32, tag="rstd8")
                nc.scalar.activation(out=rstd8[:], in_=lnv8[:], func=Act.Exp,
                                     bias=0.0, scale=-0.5)
                for t in range(NT):
                    hn = hnp.tile([P, C], bf16, tag="hn", name="hn")
                    nc.vector.tensor_scalar(out=hn[:], in0=src3[:, t, :],
                                            scalar1=mean8[:, t:t + 1],
                                            scalar2=rstd8[:, t:t + 1],
                                            op0=Alu.subtract, op1=Alu.mult)
                    nc.sync.dma_start(out=hn_dram[s, which, t * P:(t + 1) * P, :], in_=hn[:])
                for c2 in range(2):
                    nc.sync.dma_start_transpose(
                        out=hT[:, c2, :],
                        in_=hn_dram[s, which, :, c2 * P:(c2 + 1) * P])

            qkT = [None] * SPC
            vaug = [None] * SPC
            hT_keep = [None] * SPC

            # =========== per-sample prologue + attention ===========
            for s in range(SPC):
                # masks from nrc
                nrc_bc = stats.tile([P, 1], f32, tag="nrcbc")
                nc.sync.dma_start(out=nrc_bc[:], in_=nrcf_ext[s:s + 1, 0:1].to_broadcast((P, 1)))
                nc.vector.tensor_scalar(out=mval[s][:], in0=iota_sb[:], scalar1=nrc_bc[:],
                                        scalar2=None, op0=Alu.is_lt)
                nc.vector.tensor_scalar(out=minv[s][:], in0=mval[s][:], scalar1=-1.0,
                                        scalar2=1.0, op0=Alu.mult, op1=Alu.add)
                # kb = mval*( -SHIFT + ... ): valid -> SHIFT, invalid -> NEG
                nc.vector.tensor_scalar(out=kb[s][:], in0=mval[s][:], scalar1=(-NEG + SHIFT),
                                        scalar2=NEG, op0=Alu.mult, op1=Alu.add)

                nc.sync.dma_start(out=x_sb[s][:], in_=x_ext[s].rearrange("(t p) c -> p t c", p=P))
                Rv = nc.values_load(meta_sb[0:1, 2 * s:2 * s + 1], min_val=0, max_val=NT,
                                    skip_runtime_bounds_check=True)
                R2v = nc.values_load(meta_sb[0:1, 2 * s + 1:2 * s + 2], min_val=0, max_val=NT,
                                     skip_runtime_bounds_check=True)

                if parts == "w":
                    continue
                # LN1 -> hT (transposed) via DRAM round-trip
                hT = big.tile([P, 2, N], bf16, tag="hT")
                layernorm_all(x_sb[s], s, 0, hT)
                hT_keep[s] = hT

                # qkT = Wqk' @ hT   [P, 4, N] (rows: q0..q255, k0..k255 chunked by 128)
                if parts == "ln":
                    continue
                qkT[s] = big.tile([P, 4, N], bf16, tag="qkT", name=f"qkT{s}")
                def qkT_chunk(icq):
                    for r in range(4):
                        ps = psw.tile([P, ICW], f32, tag="w", name="psqk")
                        for c2 in range(2):
                            nc.tensor.matmul(ps[:], lhsT=wqk_sb[:, c2, r * P:(r + 1) * P],
                                             rhs=hT[:, c2, icq * ICW:(icq + 1) * ICW],
                                             start=(c2 == 0), stop=(c2 == 1))
                        if r < 2:
                            nc.vector.tensor_scalar(out=qkT[s][:, r, icq * ICW:(icq + 1) * ICW],
                                                    in0=ps[:], scalar1=bqk_sb[:, r:r + 1],
                                                    scalar2=None, op0=Alu.add)
                        else:
                            nc.scalar.add(out=qkT[s][:, r, icq * ICW:(icq + 1) * ICW],
                                          in_=ps[:], add=bqk_sb[:, r:r + 1])

                for icq in range(C_max):
                    qkT_chunk(icq)

                if parts == "qkt":
                    continue
                # v rows + ones col -> v_aug [P, NT, 4, 65]
                va = big.tile([P, NT, 4, D + 1], bf16, tag="vaug")
                nc.gpsimd.memset(va[:, :, :, D:D + 1], 1.0)
                for t in range(NT):
                    psv = psw.tile([P, C], f32, tag="w")
                    for c2 in range(2):
                        nc.tensor.matmul(psv[:], lhsT=hT[:, c2, t * P:(t + 1) * P],
                                         rhs=wv_sb[:, c2, :], start=(c2 == 0), stop=(c2 == 1))
                    for h in range(H):
                        nc.vector.tensor_copy(out=va[:, t, h, 0:D], in_=psv[:, h * D:(h + 1) * D])
                vaug[s] = va

                # uniform attention value u = (mean_v + bv) @ projT + pb
                psmv = psw.tile([1, H * (D + 1)], f32, tag="w")
                for t in range(NT):
                    nc.tensor.matmul(psmv[:], lhsT=ucol_sb[:], rhs=va[:, t, :, :],
                                     start=(t == 0), stop=(t == NT - 1))
                u_tmp = work.tile([1, C], f32, tag="utmp")
                for h in range(H):
                    nc.vector.tensor_copy(out=u_tmp[0:1, h * D:(h + 1) * D],
                                          in_=psmv[0:1, h * (D + 1):h * (D + 1) + D])
                nc.vector.tensor_tensor(out=u_tmp[:], in0=u_tmp[:], in1=bvrow_sb[:], op=Alu.add)
                mvT = work.tile([P, 2], bf16, tag="mvT")
                for c2 in range(2):
                    pst = psw.tile([P, 1], f32, tag="w")
                    nc.tensor.matmul(pst[:], lhsT=u_tmp[0:1, c2 * P:(c2 + 1) * P],
                                     rhs=ones1_sb[0:1, 0:1], start=True, stop=True)
                    nc.scalar.copy(out=mvT[:, c2:c2 + 1], in_=pst[:])
                psu = psw.tile([1, C], f32, tag="w")
                for c2 in range(2):
                    nc.tensor.matmul(psu[:], lhsT=mvT[:, c2:c2 + 1], rhs=proj_sb[:, c2, :],
                                     start=(c2 == 0), stop=(c2 == 1))
                u_row = work.tile([1, C], f32, tag="urow")
                nc.vector.tensor_tensor(out=u_row[:], in0=psu[:], in1=pbrow_sb[:], op=Alu.add)
                psub = psw.tile([P, C], f32, tag="w")
                nc.tensor.matmul(psub[:], lhsT=ones1_sb[:], rhs=u_row[:], start=True, stop=True)
                nc.scalar.copy(out=ub_sb[s][:], in_=psub[:])

                # dense pre-pass: x2 = x + u * (1 - m)   (in place on x_sb)
                for g in range(NT):
                    nc.vector.scalar_tensor_tensor(out=x_sb[s][:, g, :], in0=ub_sb[s][:],
                                                   scalar=minv[s][:, g:g + 1],
                                                   in1=x_sb[s][:, g, :],
                                                   op0=Alu.mult, op1=Alu.add)

                # ---- attention over valid region (per-sample dynamic tile
                # counts: Rv key-tiles; R2v gates the second query chunk) ----
                if parts == "vu":
                    continue
                for ic in range(C_max if parts != "mlp" else 0):
                    Rcond = Rv if ic == 0 else R2v
                    pT = pt2.tile([P, H, R_max, ICW], bf16, tag="pT", name="pT")
                    psav = [psacc.tile([D + 1, ICW], f32, tag=f"psav{h}", name=f"psav{h}")
                            for h in range(H)]
                    ktt_all = big1.tile([P, R_max, ICW], bf16, tag="ktt", name="ktt")

                    def prefetch(ic=ic, Rcond=Rcond, ktt_all=ktt_all):
                        for jt in range(R_max):
                            pair0 = (jt // 2) * 2
                            cond = None if (ic == 0 and pair0 == 0) else (Rcond > pair0)
                            nc.sync.dma_start(
                                out=ktt_all[:, jt, :],
                                in_=kt_ext[s, jt * P:(jt + 1) * P, ic * ICW:(ic + 1) * ICW],
                                cond=cond)

                    def jt_body(jt, ic=ic, pT=pT, psav=psav, ktt_all=ktt_all):
                        for h in range(H):
                            pss = psw.tile([P, ICW], f32, tag="w", name="pss")
                            nc.tensor.matmul(pss[:], lhsT=id_sb[:], rhs=ktt_all[:, jt, :],
                                             start=True, stop=False)
                            mo = (h % 2) * D
                            nc.tensor.matmul(pss[:],
                                             lhsT=qkT[s][mo:mo + D, 2 + h // 2, jt * P:(jt + 1) * P],
                                             rhs=qkT[s][mo:mo + D, h // 2, ic * ICW:(ic + 1) * ICW],
                                             start=False, stop=True)
                            nc.scalar.activation(out=pT[:, h, jt, :], in_=pss[:], func=Act.Exp,
                                                 bias=kb[s][:, jt:jt + 1], scale=1.0)
                        for h in range(H):
                            nc.tensor.matmul(psav[h][:], lhsT=va[:, jt, h, :],
                                             rhs=pT[:, h, jt, :],
                                             start=(jt == 0), stop=False)

                    def chunk_tail(ic=ic, psav=psav):
                        # close the (possibly branch-shortened) accumulation
                        # groups with zero-contribution matmuls
                        for h in range(H):
                            nc.tensor.matmul(psav[h][:], lhsT=zcol_sb[:], rhs=zrow_sb[:],
                                             start=False, stop=True)
                        # softmax denominators -> r = 1/s broadcast per head
                        for h in range(H):
                            if h % 2 == 0:
                                nc.vector.tensor_copy(out=stage_sb[32 * h:32 * h + 1, :],
                                                      in_=psav[h][D:D + 1, :])
                            else:
                                nc.scalar.copy(out=stage_sb[32 * h:32 * h + 1, :],
                                               in_=psav[h][D:D + 1, :])
                        r_sb = epi.tile([P, 2, ICW], f32, tag="rsb", name="rsb")
                        lntmp = epi.tile([P, ICW], f32, tag="lntmp", name="lntmp")
                        for c2 in range(2):
                            psr = psw.tile([P, ICW], f32, tag="w", name="psr")
                            nc.tensor.matmul(psr[:], lhsT=sel_sb[:, c2 * P:(c2 + 1) * P],
                                             rhs=stage_sb[:], start=True, stop=True)
                            nc.scalar.activation(out=lntmp[:], in_=psr[:], func=Act.Ln,
                                                 bias=0.0, scale=1.0)
                            nc.scalar.activation(out=r_sb[:, c2, :], in_=lntmp[:], func=Act.Exp,
                                                 bias=0.0, scale=-1.0)
                        oT = epi.tile([P, 2, ICW], bf16, tag="oT", name="oT")
                        for h in range(H):
                            mo = (h % 2) * D
                            nc.vector.tensor_tensor(out=oT[mo:mo + D, h // 2, :],
                                                    in0=psav[h][0:D, :],
                                                    in1=r_sb[mo:mo + D, h // 2, :], op=Alu.mult)
                        for c2 in range(2):
                            nc.vector.tensor_scalar(out=oT[:, c2, :], in0=oT[:, c2, :],
                                                    scalar1=bv_sb[:, c2:c2 + 1], scalar2=None,
                                                    op0=Alu.add)
                        for it in range(ICW // P):
                            g = ic * (ICW // P) + it
                            psp = psw.tile([P, C], f32, tag="w", name="psp")
                            for c2 in range(2):
                                nc.tensor.matmul(psp[:], lhsT=oT[:, c2, it * P:(it + 1) * P],
                                                 rhs=proj_sb[:, c2, :], start=(c2 == 0), stop=False)
                            nc.tensor.matmul(psp[:], lhsT=ones1_bf[:], rhs=pbrow_sb[:],
                                             start=False, stop=True)
                            nc.vector.scalar_tensor_tensor(out=x_sb[s][:, g, :], in0=psp[:],
                                                           scalar=mval[s][:, g:g + 1],
                                                           in1=x_sb[s][:, g, :],
                                                           op0=Alu.mult, op1=Alu.add)

                    def chunk(ic=ic, Rcond=Rcond):
                        prefetch()
                        for pr in range((R_max + 1) // 2):
                            jts = [jt for jt in (2 * pr, 2 * pr + 1) if jt < R_max]

                            def pair_body(jts=jts):
                                for jt in jts:
                                    jt_body(jt)

                            if ic == 0 and pr == 0:
                                pair_body()
                            else:
                                with tc.If(Rcond > 2 * pr):
                                    pair_body()
                        chunk_tail()

                    chunk()

            if parts in ("attn", "w", "ln", "qkt", "vu"):
                return_early = True
            else:
                return_early = False
            if parts in ("attn", "w", "ln", "qkt", "vu"):
                for s in range(SPC):
                    for t in range(NT):
                        nc.sync.dma_start(out=out_ext[s, t * P:(t + 1) * P, :],
                                          in_=x_sb[s][:, t, :])
            # =========== MLP phase: LN2 for both samples first (keeps the
            # exp/ln ACT table resident until all Exp work is done), then
            # fc1+gelu+fc2 per sample ===========
            h2T_keep = [None] * SPC
            for s in range(SPC if not return_early else 0):
                h2T = big.tile([P, 2, N], bf16, tag="h2T")
                layernorm_all(x_sb[s], s, 1, h2T)
                h2T_keep[s] = h2T
            for s in range(SPC if not return_early else 0):
                h2T = h2T_keep[s]
                mT = big1.tile([P, 8, N], bf16, tag="mT")
                for r in range(8):
                    for icol in range(2):
                        psf = psw.tile([P, ICW], f32, tag="w")
                        for c2 in range(2):
                            nc.tensor.matmul(psf[:], lhsT=fc1_sb[:, c2, r * P:(r + 1) * P],
                                             rhs=h2T[:, c2, icol * ICW:(icol + 1) * ICW],
                                             start=(c2 == 0), stop=(c2 == 1))
                        nc.scalar.activation(out=mT[:, r, icol * ICW:(icol + 1) * ICW],
                                             in_=psf[:], func=Act.Gelu,
                                             bias=bf1_sb[:, r:r + 1], scale=1.0)
                for t in range(NT):
                    psf2 = psw.tile([P, C], f32, tag="w")
                    for r in range(8):
                        nc.tensor.matmul(psf2[:], lhsT=mT[:, r, t * P:(t + 1) * P],
                                         rhs=fc2_sb[:, r, :], start=(r == 0), stop=False)
                    nc.tensor.matmul(psf2[:], lhsT=ones1_bf[:], rhs=f2b_sb[:],
                                     start=False, stop=True)
                    o_sb = outp.tile([P, C], f32, tag="o")
                    nc.vector.tensor_tensor(out=o_sb[:], in0=psf2[:], in1=x_sb[s][:, t, :],
                                            op=Alu.add)
                    nc.sync.dma_start(out=out_ext[s, t * P:(t + 1) * P, :], in_=o_sb[:])

    nc.finalize()
    return nc


def _prep(inputs):
    """Host-side preprocessing: sharding metadata + weight folding."""
    import ml_dtypes
    bf16 = ml_dtypes.bfloat16

    x = np.ascontiguousarray(np.asarray(inputs["x"], dtype=np.float32))
    K = np.asarray(inputs["K"], dtype=np.float32)
    n1 = np.asarray(inputs["n1"]).astype(np.int64)
    n2 = np.asarray(inputs["n2"]).astype(np.int64)
    nrc = n1 * n2
    scale = D ** -0.5

    g1 = np.asarray(inputs["ln1_g"], np.float32)
    b1 = np.asarray(inputs["ln1_b"], np.float32)
    g2 = np.asarray(inputs["ln2_g"], np.float32)
    b2 = np.asarray(inputs["ln2_b"], np.float32)
    qkv_w = np.asarray(inputs["qkv_w"], np.float32)
    qkv_b = np.asarray(inputs["qkv_b"], np.float32)

    Wqk = qkv_w[:2 * C]
    bqk = Wqk @ b1 + qkv_b[:2 * C]
    Wqk_eff = Wqk * g1[None, :]
    Wqk_eff = Wqk_eff.copy()
    Wqk_eff[:C] *= scale
    bqk = bqk.copy()
    bqk[:C] *= scale
    Wv = qkv_w[2 * C:]
    bv = Wv @ b1 + qkv_b[2 * C:]
    Wv_eff = Wv * g1[None, :]
    W1 = np.asarray(inputs["fc1_w"], np.float32)
    bf1 = W1 @ b2 + np.asarray(inputs["fc1_b"], np.float32)
    W1_eff = W1 * g2[None, :]

    sel4 = np.zeros((P, C), np.float32)
    for h in range(H):
        sel4[32 * h, h * D:(h + 1) * D] = 1.0

    shared = {
        "wqk_t": np.ascontiguousarray(Wqk_eff.T).astype(bf16),
        "bqk_col": np.ascontiguousarray(bqk.reshape(4, P).T),
        "wv_t": np.ascontiguousarray(Wv_eff.T).astype(bf16),
        "bv_col": np.ascontiguousarray(bv.reshape(2, P).T),
        "bv_row": bv.reshape(1, C).copy(),
        "proj_t": np.ascontiguousarray(np.asarray(inputs["proj_w"], np.float32).T).astype(bf16),
        "pb_row": np.asarray(inputs["proj_b"], np.float32).reshape(1, C).astype(bf16),
        "fc1_t": np.ascontiguousarray(W1_eff.T).astype(bf16),
        "bf1_col": np.ascontiguousarray(bf1.reshape(8, P).T),
        "fc2_t": np.ascontiguousarray(np.asarray(inputs["fc2_w"], np.float32).T).astype(bf16),
        "f2b_row": np.asarray(inputs["fc2_b"], np.float32).reshape(1, C).astype(bf16),
        "sel4": sel4,
        "ident": np.eye(P, dtype=np.float32).astype(bf16),
        "iota_pt": (np.arange(P, dtype=np.float32)[:, None]
                    + P * np.arange(NT, dtype=np.float32)[None, :]).copy(),
    }

    # balance: sort by nrc, pair largest with smallest
    order = np.argsort(nrc)
    pairs = [(int(order[B - 1 - i]), int(order[i])) for i in range(NCORES)]

    kt_all = np.ascontiguousarray(K.transpose(0, 2, 1)).astype(bf16)

    Rarr = ((nrc + P - 1) // P).astype(np.int32)
    Carr = ((nrc + ICW - 1) // ICW).astype(np.int32)
    in_maps = []
    for a, b in pairs:
        m = dict(shared)
        m["x"] = np.ascontiguousarray(x[[a, b]])
        m["kt"] = np.ascontiguousarray(kt_all[[a, b]])
        m["nrcf"] = nrc[[a, b]].reshape(SPC, 1).astype(np.float32)
        fl = []
        for sidx in (a, b):
            R_i = int(Rarr[sidx])
            fl += [R_i, R_i if int(Carr[sidx]) >= 2 else 0]
        m["rflags"] = np.asarray(fl, np.int32).reshape(1, 2 * SPC)
        in_maps.append(m)

    R_max = int(np.max((nrc + P - 1) // P))
    C_max = int(np.max((nrc + ICW - 1) // ICW))
    return in_maps, pairs, R_max, C_max


def kernel(**inputs):
    from concourse.bass_utils import run_bass_kernel_spmd

    in_maps, pairs, R_max, C_max = _prep(inputs)
    nc = _build(R_max, C_max)
    res = run_bass_kernel_spmd(nc, in_maps, core_ids=list(range(NCORES)), trace=False)

    out = np.empty((B, N, C), np.float32)
    for c, (a, b) in enumerate(pairs):
        got = res.results[c]["out"]
        out[a] = got[0]
        out[b] = got[1]
    return out


if __name__ == "__main__":
    import reference as R

    inp = {k: np.asarray(v) for k, v in R.setup_inputs().items()}
    got = kernel(**inp)
    import jax
    exp = np.asarray(R.reference(**inp))
    rel = np.linalg.norm(got - exp) / np.linalg.norm(exp)
    print("Relative error:", rel)


# revision 23
# speedup vs baseline: 2.7506x; 2.7506x over previous
"""Trainium2 Bass kernel for nn_AssociationLayer (sparse-attention transformer block).

Sharding: pure data-parallel over batch. B=16 samples across 8 cores, 2 samples
per core, no collectives. Host pre-transposes K and folds LN gains / q-scale
into the weight matrices; the device computes LN -> qkv -> masked attention ->
proj -> residual -> LN -> MLP -> residual per sample.

Attention math (validated vs reference): with nrc = n1*n2,
  rows i <  nrc: softmax over keys j < nrc of (q_i.k_j/sqrt(D) + K[i,j]) @ v
  rows i >= nrc: uniform attention = mean over ALL keys of v
Scores are computed transposed (S^T[j,i], keys on partitions) so the key mask
and softmax shift ride the ACT exp bias, and exp(S^T) feeds the AV matmul as
lhsT with no transposes. Row sums come from a ones-column in v_aug; 1/s is
exp(-ln(s)) on ACT.
"""

import numpy as np

B, N, C = 16, 1024, 256
H, D = 4, 64
NCORES = 8
SPC = 2  # samples per core
P = 128
NT = N // P  # 8 token tiles
ICW = 512  # query-chunk width
NEG = -1.0e10
SHIFT = -12.0  # exp stability shift
EPS = 1e-5


def _build(R_max, C_max, parts="all"):
    import concourse.bass as bass
    import concourse.mybir as mybir
    import concourse.tile as tile
    from concourse import bacc

    f32 = mybir.dt.float32
    bf16 = mybir.dt.bfloat16
    Alu = mybir.AluOpType
    Act = mybir.ActivationFunctionType

    # Force Exp/Ln to resolve to the combined natural_log_exp set so the
    # greedy table-load pass doesn't ping-pong between exp_and_others and
    # natural_log (each reload costs ~2.7us on ACT). Indices are preserved.
    import concourse.hw_specs as hw_specs
    if not getattr(bacc, "_act_tables_patched", False):
        _orig_get_tables = hw_specs.get_activation_tables

        def _patched_tables(arch):
            tabs = dict(_orig_get_tables(arch))
            for nm in list(tabs.keys()):
                if nm != "natural_log_exp_and_others":
                    tabs[nm] = set(tabs[nm]) - {Act.Exp, Act.Ln}
            return tabs

        bacc.get_activation_tables = _patched_tables
        bacc._act_tables_patched = True

    nc = bacc.Bacc()

    # ---- DRAM parameters (order = in_maps keys) ----
    x_ext = nc.declare_dram_parameter("x", [SPC, N, C], f32, isOutput=False)
    kt_ext = nc.declare_dram_parameter("kt", [SPC, N, N], bf16, isOutput=False)
    nrcf_ext = nc.declare_dram_parameter("nrcf", [SPC, 1], f32, isOutput=False)
    wqk_ext = nc.declare_dram_parameter("wqk_t", [C, 2 * C], bf16, isOutput=False)
    bqk_ext = nc.declare_dram_parameter("bqk_col", [P, 4], f32, isOutput=False)
    wv_ext = nc.declare_dram_parameter("wv_t", [C, C], bf16, isOutput=False)
    bv_ext = nc.declare_dram_parameter("bv_col", [P, 2], f32, isOutput=False)
    bvrow_ext = nc.declare_dram_parameter("bv_row", [1, C], f32, isOutput=False)
    proj_ext = nc.declare_dram_parameter("proj_t", [C, C], bf16, isOutput=False)
    pbrow_ext = nc.declare_dram_parameter("pb_row", [1, C], bf16, isOutput=False)
    fc1_ext = nc.declare_dram_parameter("fc1_t", [C, 4 * C], bf16, isOutput=False)
    bf1_ext = nc.declare_dram_parameter("bf1_col", [P, 8], f32, isOutput=False)
    fc2_ext = nc.declare_dram_parameter("fc2_t", [4 * C, C], bf16, isOutput=False)
    f2b_ext = nc.declare_dram_parameter("f2b_row", [1, C], bf16, isOutput=False)
    sel_ext = nc.declare_dram_parameter("sel4", [P, C], f32, isOutput=False)
    id_ext = nc.declare_dram_parameter("ident", [P, P], bf16, isOutput=False)
    iota_ext = nc.declare_dram_parameter("iota_pt", [P, NT], f32, isOutput=False)
    meta_ext = nc.declare_dram_parameter("rflags", [1, 2 * SPC], mybir.dt.int32, isOutput=False)
    out_ext = nc.declare_dram_parameter("out", [SPC, N, C], f32, isOutput=True)
    hn_dram = nc.dram_tensor("hn_stage", [SPC, 2, N, C], bf16)  # [s, which_ln, tokens, chan]

    with tile.TileContext(nc) as tc:
        with (
            tc.tile_pool(name="singles", bufs=1) as singles,
            tc.tile_pool(name="big", bufs=2) as big,
            tc.tile_pool(name="big1", bufs=1) as big1,
            tc.tile_pool(name="pt2", bufs=2) as pt2,
            tc.tile_pool(name="hnp", bufs=8) as hnp,
            tc.tile_pool(name="epi", bufs=2) as epi,
            tc.tile_pool(name="work", bufs=3) as work,
            tc.tile_pool(name="stats", bufs=4) as stats,
            tc.tile_pool(name="outp", bufs=3) as outp,
            tc.tile_pool(name="psw", bufs=4, space="PSUM") as psw,
            tc.tile_pool(name="psacc", bufs=1, space="PSUM") as psacc,
        ):
            # ---- constants / weights resident in SBUF ----
            wqk_sb = singles.tile([P, 2, 2 * C], bf16, tag="wqk")
            nc.sync.dma_start(out=wqk_sb[:], in_=wqk_ext.rearrange("(c2 p) r -> p c2 r", p=P))
            wv_sb = singles.tile([P, 2, C], bf16, tag="wv")
            nc.sync.dma_start(out=wv_sb[:], in_=wv_ext.rearrange("(c2 p) r -> p c2 r", p=P))
            proj_sb = singles.tile([P, 2, C], bf16, tag="proj")
            nc.sync.dma_start(out=proj_sb[:], in_=proj_ext.rearrange("(c2 p) r -> p c2 r", p=P))
            fc1_sb = singles.tile([P, 2, 4 * C], bf16, tag="fc1")
            nc.sync.dma_start(out=fc1_sb[:], in_=fc1_ext.rearrange("(c2 p) r -> p c2 r", p=P))
            fc2_sb = singles.tile([P, 8, C], bf16, tag="fc2")
            nc.sync.dma_start(out=fc2_sb[:], in_=fc2_ext.rearrange("(r p) c -> p r c", p=P))
            bqk_sb = singles.tile([P, 4], f32, tag="bqk")
            nc.sync.dma_start(out=bqk_sb[:], in_=bqk_ext[:])
            bv_sb = singles.tile([P, 2], f32, tag="bv")
            nc.sync.dma_start(out=bv_sb[:], in_=bv_ext[:])
            bvrow_sb = singles.tile([1, C], f32, tag="bvrow")
            nc.sync.dma_start(out=bvrow_sb[:], in_=bvrow_ext[:])
            pbrow_sb = singles.tile([1, C], bf16, tag="pbrow")
            nc.sync.dma_start(out=pbrow_sb[:], in_=pbrow_ext[:])
            bf1_sb = singles.tile([P, 8], f32, tag="bf1")
            nc.sync.dma_start(out=bf1_sb[:], in_=bf1_ext[:])
            f2b_sb = singles.tile([1, C], bf16, tag="f2b")
            nc.sync.dma_start(out=f2b_sb[:], in_=f2b_ext[:])
            sel_sb = singles.tile([P, C], f32, tag="sel")
            nc.sync.dma_start(out=sel_sb[:], in_=sel_ext[:])
            id_sb = singles.tile([P, P], bf16, tag="ident")
            nc.sync.dma_start(out=id_sb[:], in_=id_ext[:])
            iota_sb = singles.tile([P, NT], f32, tag="iota")
            nc.sync.dma_start(out=iota_sb[:], in_=iota_ext[:])
            meta_sb = singles.tile([1, 2 * SPC], mybir.dt.int32, tag="meta")
            nc.sync.dma_start(out=meta_sb[:], in_=meta_ext[:])
            eps_sb = singles.tile([P, 1], f32, tag="eps")
            nc.gpsimd.memset(eps_sb[:], EPS)
            ones1_sb = singles.tile([1, P], f32, tag="ones1")
            nc.gpsimd.memset(ones1_sb[:], 1.0)
            ones1_bf = singles.tile([1, P], bf16, tag="ones1bf")
            nc.gpsimd.memset(ones1_bf[:], 1.0)
            ucol_sb = singles.tile([P, 1], bf16, tag="ucol")
            nc.gpsimd.memset(ucol_sb[:], 1.0 / N)
            stage_sb = singles.tile([P, ICW], f32, tag="stage")
            nc.gpsimd.memset(stage_sb[:], 1.0)
            zcol_sb = singles.tile([1, D + 1], bf16, tag="zcol")
            nc.gpsimd.memset(zcol_sb[:], 0.0)
            zrow_sb = singles.tile([1, ICW], bf16, tag="zrow")
            nc.gpsimd.memset(zrow_sb[:], 0.0)

            # per-sample persistent tiles
            x_sb = [singles.tile([P, NT, C], f32, tag=f"x{s}", name=f"x{s}") for s in range(SPC)]
            mval = [singles.tile([P, NT], f32, tag=f"mval{s}", name=f"mval{s}") for s in range(SPC)]
            minv = [singles.tile([P, NT], f32, tag=f"minv{s}", name=f"minv{s}") for s in range(SPC)]
            kb = [singles.tile([P, NT], f32, tag=f"kb{s}", name=f"kb{s}") for s in range(SPC)]
            ub_sb = [singles.tile([P, C], f32, tag=f"ub{s}", name=f"ub{s}") for s in range(SPC)]

            def layernorm_all(src3, s, which, hT):
                """LN over free dim C of all NT tiles of src3 -> hT [P, 2, N]
                (transposed, hT[c, i]) via a DRAM round-trip: 8 row DMAs out,
                2 large xbar transposes back. rstd batched in one Ln+Exp."""
                mean8 = stats.tile([P, NT], f32, tag="mean8")
                var8 = stats.tile([P, NT], f32, tag="var8")
                for t in range(NT):
                    st6 = stats.tile([P, 6], f32, tag="st6")
                    nc.vector.bn_stats(out=st6[:], in_=src3[:, t, :])
                    mv2 = stats.tile([P, 2], f32, tag="mv2")
                    nc.vector.bn_aggr(out=mv2[:], in_=st6[:])
                    nc.vector.tensor_copy(out=mean8[:, t:t + 1], in_=mv2[:, 0:1])
                    nc.vector.tensor_copy(out=var8[:, t:t + 1], in_=mv2[:, 1:2])
                lnv8 = stats.tile([P, NT], f32, tag="lnv8")
                nc.scalar.activation(out=lnv8[:], in_=var8[:], func=Act.Ln,
                                     bias=eps_sb[:], scale=1.0)
                rstd8 = stats.tile([P, NT], f32, tag="rstd8")
                nc.scalar.activation(out=rstd8[:], in_=lnv8[:], func=Act.Exp,
                                     bias=0.0, scale=-0.5)
                for t in range(NT):
                    hn = hnp.tile([P, C], bf16, tag="hn", name="hn")
                    nc.vector.tensor_scalar(out=hn[:], in0=src3[:, t, :],
                                            scalar1=mean8[:, t:t + 1],
                                            scalar2=rstd8[:, t:t + 1],
                                            op0=Alu.subtract, op1=Alu.mult)
                    nc.sync.dma_start(out=hn_dram[s, which, t * P:(t + 1) * P, :], in_=hn[:])
                for c2 in range(2):
                    nc.sync.dma_start_transpose(
                        out=hT[:, c2, :],
                        in_=hn_dram[s, which, :, c2 * P:(c2 + 1) * P])

            qkT = [None] * SPC
            vaug = [None] * SPC
            hT_keep = [None] * SPC

            # =========== per-sample prologue + attention ===========
            for s in range(SPC):
                # masks from nrc
                nrc_bc = stats.tile([P, 1], f32, tag="nrcbc")
                nc.sync.dma_start(out=nrc_bc[:], in_=nrcf_ext[s:s + 1, 0:1].to_broadcast((P, 1)))
                nc.vector.tensor_scalar(out=mval[s][:], in0=iota_sb[:], scalar1=nrc_bc[:],
                                        scalar2=None, op0=Alu.is_lt)
                nc.vector.tensor_scalar(out=minv[s][:], in0=mval[s][:], scalar1=-1.0,
                                        scalar2=1.0, op0=Alu.mult, op1=Alu.add)
                # kb = mval*( -SHIFT + ... ): valid -> SHIFT, invalid -> NEG
                nc.vector.tensor_scalar(out=kb[s][:], in0=mval[s][:], scalar1=(-NEG + SHIFT),
                                        scalar2=NEG, op0=Alu.mult, op1=Alu.add)

                nc.sync.dma_start(out=x_sb[s][:], in_=x_ext[s].rearrange("(t p) c -> p t c", p=P))
                Rv = nc.values_load(meta_sb[0:1, 2 * s:2 * s + 1], min_val=0, max_val=NT,
                                    skip_runtime_bounds_check=True)
                R2v = nc.values_load(meta_sb[0:1, 2 * s + 1:2 * s + 2], min_val=0, max_val=NT,
                                     skip_runtime_bounds_check=True)

                if parts == "w":
                    continue
                # LN1 -> hT (transposed) via DRAM round-trip
                hT = big.tile([P, 2, N], bf16, tag="hT")
                layernorm_all(x_sb[s], s, 0, hT)
                hT_keep[s] = hT

                # qkT = Wqk' @ hT   [P, 4, N] (rows: q0..q255, k0..k255 chunked by 128)
                if parts == "ln":
                    continue
                qkT[s] = big.tile([P, 4, N], bf16, tag="qkT", name=f"qkT{s}")
                def qkT_chunk(icq):
                    for r in range(4):
                        ps = psw.tile([P, ICW], f32, tag="w", name="psqk")
                        for c2 in range(2):
                            nc.tensor.matmul(ps[:], lhsT=wqk_sb[:, c2, r * P:(r + 1) * P],
                                             rhs=hT[:, c2, icq * ICW:(icq + 1) * ICW],
                                             start=(c2 == 0), stop=(c2 == 1))
                        if r < 2:
                            nc.vector.tensor_scalar(out=qkT[s][:, r, icq * ICW:(icq + 1) * ICW],
                                                    in0=ps[:], scalar1=bqk_sb[:, r:r + 1],
                                                    scalar2=None, op0=Alu.add)
                        else:
                            nc.scalar.add(out=qkT[s][:, r, icq * ICW:(icq + 1) * ICW],
                                          in_=ps[:], add=bqk_sb[:, r:r + 1])

                qkT_chunk(0)
                for icq in range(1, C_max):
                    with tc.If(R2v > 0):
                        qkT_chunk(icq)

                if parts == "qkt":
                    continue
                # v rows + ones col -> v_aug [P, NT, 4, 65]
                va = big.tile([P, NT, 4, D + 1], bf16, tag="vaug")
                nc.gpsimd.memset(va[:, :, :, D:D + 1], 1.0)
                for t in range(NT):
                    psv = psw.tile([P, C], f32, tag="w")
                    for c2 in range(2):
                        nc.tensor.matmul(psv[:], lhsT=hT[:, c2, t * P:(t + 1) * P],
                                         rhs=wv_sb[:, c2, :], start=(c2 == 0), stop=(c2 == 1))
                    for h in range(H):
                        nc.vector.tensor_copy(out=va[:, t, h, 0:D], in_=psv[:, h * D:(h + 1) * D])
                vaug[s] = va

                # uniform attention value u = (mean_v + bv) @ projT + pb
                psmv = psw.tile([1, H * (D + 1)], f32, tag="w")
                for t in range(NT):
                    nc.tensor.matmul(psmv[:], lhsT=ucol_sb[:], rhs=va[:, t, :, :],
                                     start=(t == 0), stop=(t == NT - 1))
                u_tmp = work.tile([1, C], f32, tag="utmp")
                for h in range(H):
                    nc.vector.tensor_copy(out=u_tmp[0:1, h * D:(h + 1) * D],
                                          in_=psmv[0:1, h * (D + 1):h * (D + 1) + D])
                nc.vector.tensor_tensor(out=u_tmp[:], in0=u_tmp[:], in1=bvrow_sb[:], op=Alu.add)
                mvT = work.tile([P, 2], bf16, tag="mvT")
                for c2 in range(2):
                    pst = psw.tile([P, 1], f32, tag="w")
                    nc.tensor.matmul(pst[:], lhsT=u_tmp[0:1, c2 * P:(c2 + 1) * P],
                                     rhs=ones1_sb[0:1, 0:1], start=True, stop=True)
                    nc.scalar.copy(out=mvT[:, c2:c2 + 1], in_=pst[:])
                psu = psw.tile([1, C], f32, tag="w")
                for c2 in range(2):
                    nc.tensor.matmul(psu[:], lhsT=mvT[:, c2:c2 + 1], rhs=proj_sb[:, c2, :],
                                     start=(c2 == 0), stop=(c2 == 1))
                u_row = work.tile([1, C], f32, tag="urow")
                nc.vector.tensor_tensor(out=u_row[:], in0=psu[:], in1=pbrow_sb[:], op=Alu.add)
                psub = psw.tile([P, C], f32, tag="w")
                nc.tensor.matmul(psub[:], lhsT=ones1_sb[:], rhs=u_row[:], start=True, stop=True)
                nc.scalar.copy(out=ub_sb[s][:], in_=psub[:])

                # dense pre-pass: x2 = x + u * (1 - m)   (in place on x_sb)
                for g in range(NT):
                    nc.vector.scalar_tensor_tensor(out=x_sb[s][:, g, :], in0=ub_sb[s][:],
                                                   scalar=minv[s][:, g:g + 1],
                                                   in1=x_sb[s][:, g, :],
                                                   op0=Alu.mult, op1=Alu.add)

                # ---- attention over valid region (per-sample dynamic tile
                # counts: Rv key-tiles; R2v gates the second query chunk) ----
                if parts == "vu":
                    continue
                for ic in range(C_max if parts != "mlp" else 0):
                    Rcond = Rv if ic == 0 else R2v
                    pT = pt2.tile([P, H, R_max, ICW], bf16, tag="pT", name="pT")
                    psav = [psacc.tile([D + 1, ICW], f32, tag=f"psav{h}", name=f"psav{h}")
                            for h in range(H)]
                    ktt_all = big1.tile([P, R_max, ICW], bf16, tag="ktt", name="ktt")

                    def prefetch(ic=ic, Rcond=Rcond, ktt_all=ktt_all):
                        for jt in range(R_max):
                            pair0 = (jt // 2) * 2
                            cond = None if (ic == 0 and pair0 == 0) else (Rcond > pair0)
                            nc.sync.dma_start(
                                out=ktt_all[:, jt, :],
                                in_=kt_ext[s, jt * P:(jt + 1) * P, ic * ICW:(ic + 1) * ICW],
                                cond=cond)

                    def jt_body(jt, ic=ic, pT=pT, psav=psav, ktt_all=ktt_all):
                        for h in range(H):
                            pss = psw.tile([P, ICW], f32, tag="w", name="pss")
                            nc.tensor.matmul(pss[:], lhsT=id_sb[:], rhs=ktt_all[:, jt, :],
                                             start=True, stop=False)
                            mo = (h % 2) * D
                            nc.tensor.matmul(pss[:],
                                             lhsT=qkT[s][mo:mo + D, 2 + h // 2, jt * P:(jt + 1) * P],
                                             rhs=qkT[s][mo:mo + D, h // 2, ic * ICW:(ic + 1) * ICW],
                                             start=False, stop=True)
                            nc.scalar.activation(out=pT[:, h, jt, :], in_=pss[:], func=Act.Exp,
                                                 bias=kb[s][:, jt:jt + 1], scale=1.0)
                        for h in range(H):
                            nc.tensor.matmul(psav[h][:], lhsT=va[:, jt, h, :],
                                             rhs=pT[:, h, jt, :],
                                             start=(jt == 0), stop=False,
                                             skip_group_check=True)

                    def chunk_tail(ic=ic, psav=psav):
                        # close the (possibly branch-shortened) accumulation
                        # groups with zero-contribution matmuls
                        for h in range(H):
                            nc.tensor.matmul(psav[h][:], lhsT=zcol_sb[:], rhs=zrow_sb[:],
                                             start=False, stop=True, skip_group_check=True)
                        # softmax denominators -> r = 1/s broadcast per head
                        for h in range(H):
                            if h % 2 == 0:
                                nc.vector.tensor_copy(out=stage_sb[32 * h:32 * h + 1, :],
                                                      in_=psav[h][D:D + 1, :])
                            else:
                                nc.scalar.copy(out=stage_sb[32 * h:32 * h + 1, :],
                                               in_=psav[h][D:D + 1, :])
                        r_sb = epi.tile([P, 2, ICW], f32, tag="rsb", name="rsb")
                        lntmp = epi.tile([P, ICW], f32, tag="lntmp", name="lntmp")
                        for c2 in range(2):
                            psr = psw.tile([P, ICW], f32, tag="w", name="psr")
                            nc.tensor.matmul(psr[:], lhsT=sel_sb[:, c2 * P:(c2 + 1) * P],
                                             rhs=stage_sb[:], start=True, stop=True)
                            nc.scalar.activation(out=lntmp[:], in_=psr[:], func=Act.Ln,
                                                 bias=0.0, scale=1.0)
                            nc.scalar.activation(out=r_sb[:, c2, :], in_=lntmp[:], func=Act.Exp,
                                                 bias=0.0, scale=-1.0)
                        oT = epi.tile([P, 2, ICW], bf16, tag="oT", name="oT")
                        for h in range(H):
                            mo = (h % 2) * D
                            nc.vector.tensor_tensor(out=oT[mo:mo + D, h // 2, :],
                                                    in0=psav[h][0:D, :],
                                                    in1=r_sb[mo:mo + D, h // 2, :], op=Alu.mult)
                        for c2 in range(2):
                            nc.vector.tensor_scalar(out=oT[:, c2, :], in0=oT[:, c2, :],
                                                    scalar1=bv_sb[:, c2:c2 + 1], scalar2=None,
                                                    op0=Alu.add)
                        for it in range(ICW // P):
                            g = ic * (ICW // P) + it
                            psp = psw.tile([P, C], f32, tag="w", name="psp")
                            for c2 in range(2):
                                nc.tensor.matmul(psp[:], lhsT=oT[:, c2, it * P:(it + 1) * P],
                                                 rhs=proj_sb[:, c2, :], start=(c2 == 0), stop=False)
                            nc.tensor.matmul(psp[:], lhsT=ones1_bf[:], rhs=pbrow_sb[:],
                                             start=False, stop=True)
                            nc.vector.scalar_tensor_tensor(out=x_sb[s][:, g, :], in0=psp[:],
                                                           scalar=mval[s][:, g:g + 1],
                                                           in1=x_sb[s][:, g, :],
                                                           op0=Alu.mult, op1=Alu.add)

                    def chunk(ic=ic, Rcond=Rcond):
                        for pr in range((R_max + 1) // 2):
                            jts = [jt for jt in (2 * pr, 2 * pr + 1) if jt < R_max]

                            def pair_body(jts=jts):
                                for jt in jts:
                                    jt_body(jt)

                            if ic == 0 and pr == 0:
                                pair_body()
                            else:
                                with tc.If(Rcond > 2 * pr):
                                    pair_body()
                        chunk_tail()

                    prefetch()
                    if ic == 0:
                        chunk()
                    else:
                        with tc.If(R2v > 0):
                            chunk()

            if parts in ("attn", "w", "ln", "qkt", "vu"):
                return_early = True
            else:
                return_early = False
            if parts in ("attn", "w", "ln", "qkt", "vu"):
                for s in range(SPC):
                    for t in range(NT):
                        nc.sync.dma_start(out=out_ext[s, t * P:(t + 1) * P, :],
                                          in_=x_sb[s][:, t, :])
            # =========== MLP phase: LN2 for both samples first (keeps the
            # exp/ln ACT table resident until all Exp work is done), then
            # fc1+gelu+fc2 per sample ===========
            h2T_keep = [None] * SPC
            for s in range(SPC if not return_early else 0):
                h2T = big.tile([P, 2, N], bf16, tag="h2T")
                layernorm_all(x_sb[s], s, 1, h2T)
                h2T_keep[s] = h2T
            for s in range(SPC if not return_early else 0):
                h2T = h2T_keep[s]
                mT = big1.tile([P, 8, N], bf16, tag="mT")
                for r in range(8):
                    for icol in range(2):
                        psf = psw.tile([P, ICW], f32, tag="w")
                        for c2 in range(2):
                            nc.tensor.matmul(psf[:], lhsT=fc1_sb[:, c2, r * P:(r + 1) * P],
                                             rhs=h2T[:, c2, icol * ICW:(icol + 1) * ICW],
                                             start=(c2 == 0), stop=(c2 == 1))
                        nc.scalar.activation(out=mT[:, r, icol * ICW:(icol + 1) * ICW],
                                             in_=psf[:], func=Act.Gelu,
                                             bias=bf1_sb[:, r:r + 1], scale=1.0)
                for t in range(NT):
                    psf2 = psw.tile([P, C], f32, tag="w")
                    for r in range(8):
                        nc.tensor.matmul(psf2[:], lhsT=mT[:, r, t * P:(t + 1) * P],
                                         rhs=fc2_sb[:, r, :], start=(r == 0), stop=False)
                    nc.tensor.matmul(psf2[:], lhsT=ones1_bf[:], rhs=f2b_sb[:],
                                     start=False, stop=True)
                    o_sb = outp.tile([P, C], f32, tag="o")
                    nc.vector.tensor_tensor(out=o_sb[:], in0=psf2[:], in1=x_sb[s][:, t, :],
                                            op=Alu.add)
                    nc.sync.dma_start(out=out_ext[s, t * P:(t + 1) * P, :], in_=o_sb[:])

    nc.finalize()
    return nc


def _prep(inputs):
    """Host-side preprocessing: sharding metadata + weight folding."""
    import ml_dtypes
    bf16 = ml_dtypes.bfloat16

    x = np.ascontiguousarray(np.asarray(inputs["x"], dtype=np.float32))
    K = np.asarray(inputs["K"], dtype=np.float32)
    n1 = np.asarray(inputs["n1"]).astype(np.int64)
    n2 = np.asarray(inputs["n2"]).astype(np.int64)
    nrc = n1 * n2
    scale = D ** -0.5

    g1 = np.asarray(inputs["ln1_g"], np.float32)
    b1 = np.asarray(inputs["ln1_b"], np.float32)
    g2 = np.asarray(inputs["ln2_g"], np.float32)
    b2 = np.asarray(inputs["ln2_b"], np.float32)
    qkv_w = np.asarray(inputs["qkv_w"], np.float32)
    qkv_b = np.asarray(inputs["qkv_b"], np.float32)

    Wqk = qkv_w[:2 * C]
    bqk = Wqk @ b1 + qkv_b[:2 * C]
    Wqk_eff = Wqk * g1[None, :]
    Wqk_eff = Wqk_eff.copy()
    Wqk_eff[:C] *= scale
    bqk = bqk.copy()
    bqk[:C] *= scale
    Wv = qkv_w[2 * C:]
    bv = Wv @ b1 + qkv_b[2 * C:]
    Wv_eff = Wv * g1[None, :]
    W1 = np.asarray(inputs["fc1_w"], np.float32)
    bf1 = W1 @ b2 + np.asarray(inputs["fc1_b"], np.float32)
    W1_eff = W1 * g2[None, :]

    sel4 = np.zeros((P, C), np.float32)
    for h in range(H):
        sel4[32 * h, h * D:(h + 1) * D] = 1.0

    shared = {
        "wqk_t": np.ascontiguousarray(Wqk_eff.T).astype(bf16),
        "bqk_col": np.ascontiguousarray(bqk.reshape(4, P).T),
        "wv_t": np.ascontiguousarray(Wv_eff.T).astype(bf16),
        "bv_col": np.ascontiguousarray(bv.reshape(2, P).T),
        "bv_row": bv.reshape(1, C).copy(),
        "proj_t": np.ascontiguousarray(np.asarray(inputs["proj_w"], np.float32).T).astype(bf16),
        "pb_row": np.asarray(inputs["proj_b"], np.float32).reshape(1, C).astype(bf16),
        "fc1_t": np.ascontiguousarray(W1_eff.T).astype(bf16),
        "bf1_col": np.ascontiguousarray(bf1.reshape(8, P).T),
        "fc2_t": np.ascontiguousarray(np.asarray(inputs["fc2_w"], np.float32).T).astype(bf16),
        "f2b_row": np.asarray(inputs["fc2_b"], np.float32).reshape(1, C).astype(bf16),
        "sel4": sel4,
        "ident": np.eye(P, dtype=np.float32).astype(bf16),
        "iota_pt": (np.arange(P, dtype=np.float32)[:, None]
                    + P * np.arange(NT, dtype=np.float32)[None, :]).copy(),
    }

    # balance: sort by nrc, pair largest with smallest
    order = np.argsort(nrc)
    pairs = [(int(order[B - 1 - i]), int(order[i])) for i in range(NCORES)]

    kt_all = np.ascontiguousarray(K.transpose(0, 2, 1)).astype(bf16)

    Rarr = ((nrc + P - 1) // P).astype(np.int32)
    Carr = ((nrc + ICW - 1) // ICW).astype(np.int32)
    in_maps = []
    for a, b in pairs:
        m = dict(shared)
        m["x"] = np.ascontiguousarray(x[[a, b]])
        m["kt"] = np.ascontiguousarray(kt_all[[a, b]])
        m["nrcf"] = nrc[[a, b]].reshape(SPC, 1).astype(np.float32)
        fl = []
        for sidx in (a, b):
            R_i = int(Rarr[sidx])
            fl += [R_i, R_i if int(Carr[sidx]) >= 2 else 0]
        m["rflags"] = np.asarray(fl, np.int32).reshape(1, 2 * SPC)
        in_maps.append(m)

    R_max = int(np.max((nrc + P - 1) // P))
    C_max = int(np.max((nrc + ICW - 1) // ICW))
    return in_maps, pairs, R_max, C_max


def kernel(**inputs):
    from concourse.bass_utils import run_bass_kernel_spmd

    in_maps, pairs, R_max, C_max = _prep(inputs)
    nc = _build(R_max, C_max)
    res = run_bass_kernel_spmd(nc, in_maps, core_ids=list(range(NCORES)), trace=False)

    out = np.empty((B, N, C), np.float32)
    for c, (a, b) in enumerate(pairs):
        got = res.results[c]["out"]
        out[a] = got[0]
        out[b] = got[1]
    return out


if __name__ == "__main__":
    import reference as R

    inp = {k: np.asarray(v) for k, v in R.setup_inputs().items()}
    got = kernel(**inp)
    import jax
    exp = np.asarray(R.reference(**inp))
    rel = np.linalg.norm(got - exp) / np.linalg.norm(exp)
    print("Relative error:", rel)


# revision 31
# speedup vs baseline: 4.1233x; 1.4991x over previous
"""Trainium2 Bass kernel for nn_AssociationLayer (sparse-attention transformer block).

Sharding: pure data-parallel over batch. B=16 samples across 8 cores, 2 samples
per core, no collectives. Host pre-transposes K and folds LN gains / q-scale
into the weight matrices; the device computes LN -> qkv -> masked attention ->
proj -> residual -> LN -> MLP -> residual per sample.

Attention math (validated vs reference): with nrc = n1*n2,
  rows i <  nrc: softmax over keys j < nrc of (q_i.k_j/sqrt(D) + K[i,j]) @ v
  rows i >= nrc: uniform attention = mean over ALL keys of v
Scores are computed transposed (S^T[j,i], keys on partitions) so the key mask
and softmax shift ride the ACT exp bias, and exp(S^T) feeds the AV matmul as
lhsT with no transposes. Row sums come from a ones-column in v_aug; 1/s is
exp(-ln(s)) on ACT.
"""

import numpy as np

B, N, C = 16, 1024, 256
H, D = 4, 64
NCORES = 8
SPC = 2  # samples per core
P = 128
NT = N // P  # 8 token tiles
ICW = 512  # query-chunk width
NEG = -1.0e10
SHIFT = -12.0  # exp stability shift
EPS = 1e-5


def _build(R_max, C_max, parts="all"):
    import concourse.bass as bass
    import concourse.mybir as mybir
    import concourse.tile as tile
    from concourse import bacc

    f32 = mybir.dt.float32
    bf16 = mybir.dt.bfloat16
    Alu = mybir.AluOpType
    Act = mybir.ActivationFunctionType

    # Force Exp/Ln to resolve to the combined natural_log_exp set so the
    # greedy table-load pass doesn't ping-pong between exp_and_others and
    # natural_log (each reload costs ~2.7us on ACT). Indices are preserved.
    import concourse.hw_specs as hw_specs
    if not getattr(bacc, "_act_tables_patched", False):
        _orig_get_tables = hw_specs.get_activation_tables

        def _patched_tables(arch):
            tabs = dict(_orig_get_tables(arch))
            for nm in list(tabs.keys()):
                if nm != "natural_log_exp_and_others":
                    tabs[nm] = set(tabs[nm]) - {Act.Exp, Act.Ln}
            return tabs

        bacc.get_activation_tables = _patched_tables
        bacc._act_tables_patched = True

    nc = bacc.Bacc()

    # ---- DRAM parameters (order = in_maps keys) ----
    x_ext = nc.declare_dram_parameter("x", [SPC, N, C], f32, isOutput=False)
    kt_ext = nc.declare_dram_parameter("kt", [SPC, N, N], bf16, isOutput=False)
    # small constants packed into two blobs: per-parameter NEFF binding costs
    # ~27us/exec, so fewer parameters = faster executions
    NF32 = 512 + 256 + C + 1024 + P * C + P * NT + SPC
    NBF = (C * 2 * C) + (C * C) + (C * C) + C + (C * 4 * C) + (4 * C * C) + C + (P * P)
    cf_ext = nc.declare_dram_parameter("cf32", [1, NF32], f32, isOutput=False)
    cb_ext = nc.declare_dram_parameter("cbf16", [1, NBF], bf16, isOutput=False)
    meta_ext = nc.declare_dram_parameter("rflags", [1, 2 * SPC], mybir.dt.int32, isOutput=False)
    out_ext = nc.declare_dram_parameter("out", [SPC, N, C], f32, isOutput=True)

    def _slicer(ext, sizes):
        offs = {}
        o = 0
        for nm, sz in sizes:
            offs[nm] = (o, sz)
            o += sz
        return lambda nm: ext[0, offs[nm][0]:offs[nm][0] + offs[nm][1]]
    f32_get = _slicer(cf_ext, [
        ("bqk_col", 512), ("bv_col", 256), ("bv_row", C), ("bf1_col", 1024),
        ("sel4", P * C), ("iota_pt", P * NT), ("nrcf", SPC)])
    bf_get = _slicer(cb_ext, [
        ("wqk_t", C * 2 * C), ("wv_t", C * C), ("proj_t", C * C), ("pb_row", C),
        ("fc1_t", C * 4 * C), ("fc2_t", 4 * C * C), ("f2b_row", C), ("ident", P * P)])
    hn_dram = nc.dram_tensor("hn_stage", [SPC, 2, N, C], bf16)  # [s, which_ln, tokens, chan]

    with tile.TileContext(nc) as tc:
        with (
            tc.tile_pool(name="singles", bufs=1) as singles,
            tc.tile_pool(name="big", bufs=2) as big,
            tc.tile_pool(name="big1", bufs=1) as big1,
            tc.tile_pool(name="pt2", bufs=2) as pt2,
            tc.tile_pool(name="hnp", bufs=8) as hnp,
            tc.tile_pool(name="epi", bufs=2) as epi,
            tc.tile_pool(name="work", bufs=3) as work,
            tc.tile_pool(name="stats", bufs=4) as stats,
            tc.tile_pool(name="outp", bufs=3) as outp,
            tc.tile_pool(name="psw", bufs=4, space="PSUM") as psw,
            tc.tile_pool(name="psacc", bufs=1, space="PSUM") as psacc,
        ):
            # ---- constants / weights resident in SBUF ----
            wqk_sb = singles.tile([P, 2, 2 * C], bf16, tag="wqk")
            nc.sync.dma_start(out=wqk_sb[:], in_=bf_get("wqk_t").rearrange("(c2 p r) -> p c2 r", p=P, r=2 * C))
            wv_sb = singles.tile([P, 2, C], bf16, tag="wv")
            nc.sync.dma_start(out=wv_sb[:], in_=bf_get("wv_t").rearrange("(c2 p r) -> p c2 r", p=P, r=C))
            proj_sb = singles.tile([P, 2, C], bf16, tag="proj")
            nc.sync.dma_start(out=proj_sb[:], in_=bf_get("proj_t").rearrange("(c2 p r) -> p c2 r", p=P, r=C))
            fc1_sb = singles.tile([P, 2, 4 * C], bf16, tag="fc1")
            nc.sync.dma_start(out=fc1_sb[:], in_=bf_get("fc1_t").rearrange("(c2 p r) -> p c2 r", p=P, r=4 * C))
            fc2_sb = singles.tile([P, 8, C], bf16, tag="fc2")
            nc.sync.dma_start(out=fc2_sb[:], in_=bf_get("fc2_t").rearrange("(r p c) -> p r c", p=P, c=C))
            bqk_sb = singles.tile([P, 4], f32, tag="bqk")
            nc.sync.dma_start(out=bqk_sb[:], in_=f32_get("bqk_col").rearrange("(p k) -> p k", p=P))
            bv_sb = singles.tile([P, 2], f32, tag="bv")
            nc.sync.dma_start(out=bv_sb[:], in_=f32_get("bv_col").rearrange("(p k) -> p k", p=P))
            bvrow_sb = singles.tile([1, C], f32, tag="bvrow")
            nc.sync.dma_start(out=bvrow_sb[:], in_=f32_get("bv_row").rearrange("(p k) -> p k", p=1))
            pbrow_sb = singles.tile([1, C], bf16, tag="pbrow")
            nc.sync.dma_start(out=pbrow_sb[:], in_=bf_get("pb_row").rearrange("(p k) -> p k", p=1))
            bf1_sb = singles.tile([P, 8], f32, tag="bf1")
            nc.sync.dma_start(out=bf1_sb[:], in_=f32_get("bf1_col").rearrange("(p k) -> p k", p=P))
            f2b_sb = singles.tile([1, C], bf16, tag="f2b")
            nc.sync.dma_start(out=f2b_sb[:], in_=bf_get("f2b_row").rearrange("(p k) -> p k", p=1))
            sel_sb = singles.tile([P, C], f32, tag="sel")
            nc.sync.dma_start(out=sel_sb[:], in_=f32_get("sel4").rearrange("(p k) -> p k", p=P))
            id_sb = singles.tile([P, P], bf16, tag="ident")
            nc.sync.dma_start(out=id_sb[:], in_=bf_get("ident").rearrange("(p k) -> p k", p=P))
            iota_sb = singles.tile([P, NT], f32, tag="iota")
            nc.sync.dma_start(out=iota_sb[:], in_=f32_get("iota_pt").rearrange("(p k) -> p k", p=P))
            meta_sb = singles.tile([1, 2 * SPC], mybir.dt.int32, tag="meta")
            nc.sync.dma_start(out=meta_sb[:], in_=meta_ext[:])
            eps_sb = singles.tile([P, 1], f32, tag="eps")
            nc.gpsimd.memset(eps_sb[:], EPS)
            ones1_sb = singles.tile([1, P], f32, tag="ones1")
            nc.gpsimd.memset(ones1_sb[:], 1.0)
            ones1_bf = singles.tile([1, P], bf16, tag="ones1bf")
            nc.gpsimd.memset(ones1_bf[:], 1.0)
            ucol_sb = singles.tile([P, 1], bf16, tag="ucol")
            nc.gpsimd.memset(ucol_sb[:], 1.0 / N)
            stage_sb = singles.tile([P, ICW], f32, tag="stage")
            nc.gpsimd.memset(stage_sb[:], 1.0)
            zcol_sb = singles.tile([1, D + 1], bf16, tag="zcol")
            nc.gpsimd.memset(zcol_sb[:], 0.0)
            zrow_sb = singles.tile([1, ICW], bf16, tag="zrow")
            nc.gpsimd.memset(zrow_sb[:], 0.0)

            # per-sample persistent tiles
            x_sb = [singles.tile([P, NT, C], f32, tag=f"x{s}", name=f"x{s}") for s in range(SPC)]
            mval = [singles.tile([P, NT], f32, tag=f"mval{s}", name=f"mval{s}") for s in range(SPC)]
            minv = [singles.tile([P, NT], f32, tag=f"minv{s}", name=f"minv{s}") for s in range(SPC)]
            kb = [singles.tile([P, NT], f32, tag=f"kb{s}", name=f"kb{s}") for s in range(SPC)]
            ub_sb = [singles.tile([P, C], f32, tag=f"ub{s}", name=f"ub{s}") for s in range(SPC)]

            def layernorm_all(src3, s, which, hT):
                """LN over free dim C of all NT tiles of src3 -> hT [P, 2, N]
                (transposed, hT[c, i]) via a DRAM round-trip: 8 row DMAs out,
                2 large xbar transposes back. rstd batched in one Ln+Exp."""
                mv8 = stats.tile([P, 2, NT], f32, tag="mv8")
                for t in range(NT):
                    st6 = stats.tile([P, 6], f32, tag="st6")
                    nc.vector.bn_stats(out=st6[:], in_=src3[:, t, :])
                    nc.vector.bn_aggr(out=mv8[:, :, t], in_=st6[:])
                lnv8 = stats.tile([P, NT], f32, tag="lnv8")
                nc.scalar.activation(out=lnv8[:], in_=mv8[:, 1, :], func=Act.Ln,
                                     bias=eps_sb[:], scale=1.0)
                rstd8 = stats.tile([P, NT], f32, tag="rstd8")
                nc.scalar.activation(out=rstd8[:], in_=lnv8[:], func=Act.Exp,
                                     bias=0.0, scale=-0.5)
                for t in range(NT):
                    hn = hnp.tile([P, C], bf16, tag="hn", name="hn")
                    nc.vector.tensor_scalar(out=hn[:], in0=src3[:, t, :],
                                            scalar1=mv8[:, 0, t:t + 1],
                                            scalar2=rstd8[:, t:t + 1],
                                            op0=Alu.subtract, op1=Alu.mult)
                    for c2 in range(2):
                        nc.sync.dma_start_transpose(
                            out=hT[:, c2, t * P:(t + 1) * P],
                            in_=hn[:, c2 * P:(c2 + 1) * P])

            qkT = [None] * SPC
            vaug = [None] * SPC
            hT_keep = [None] * SPC

            # =========== per-sample prologue + attention ===========
            for s in range(SPC):
                # masks from nrc
                nrc_bc = stats.tile([P, 1], f32, tag="nrcbc")
                nc.sync.dma_start(out=nrc_bc[:], in_=f32_get("nrcf")[s:s + 1].to_broadcast((P, 1)))
                nc.vector.tensor_scalar(out=mval[s][:], in0=iota_sb[:], scalar1=nrc_bc[:],
                                        scalar2=None, op0=Alu.is_lt)
                nc.vector.tensor_scalar(out=minv[s][:], in0=mval[s][:], scalar1=-1.0,
                                        scalar2=1.0, op0=Alu.mult, op1=Alu.add)
                # kb = mval*( -SHIFT + ... ): valid -> SHIFT, invalid -> NEG
                nc.vector.tensor_scalar(out=kb[s][:], in0=mval[s][:], scalar1=(-NEG + SHIFT),
                                        scalar2=NEG, op0=Alu.mult, op1=Alu.add)

                nc.sync.dma_start(out=x_sb[s][:], in_=x_ext[s].rearrange("(t p) c -> p t c", p=P))
                Rv = nc.values_load(meta_sb[0:1, 2 * s:2 * s + 1], min_val=0, max_val=NT,
                                    skip_runtime_bounds_check=True)
                R2v = nc.values_load(meta_sb[0:1, 2 * s + 1:2 * s + 2], min_val=0, max_val=NT,
                                     skip_runtime_bounds_check=True)

                if parts == "w":
                    continue
                # LN1 -> hT (transposed) via DRAM round-trip
                hT = big.tile([P, 2, N], bf16, tag="hT")
                layernorm_all(x_sb[s], s, 0, hT)
                hT_keep[s] = hT

                # qkT = Wqk' @ hT   [P, 4, N] (rows: q0..q255, k0..k255 chunked by 128)
                if parts == "ln":
                    continue
                qkT[s] = big.tile([P, 4, N], bf16, tag="qkT", name=f"qkT{s}")
                def qkT_chunk(icq):
                    for r in range(4):
                        ps = psw.tile([P, ICW], f32, tag="w", name="psqk")
                        for c2 in range(2):
                            nc.tensor.matmul(ps[:], lhsT=wqk_sb[:, c2, r * P:(r + 1) * P],
                                             rhs=hT[:, c2, icq * ICW:(icq + 1) * ICW],
                                             start=(c2 == 0), stop=(c2 == 1))
                        if r < 2:
                            nc.vector.tensor_scalar(out=qkT[s][:, r, icq * ICW:(icq + 1) * ICW],
                                                    in0=ps[:], scalar1=bqk_sb[:, r:r + 1],
                                                    scalar2=None, op0=Alu.add)
                        else:
                            nc.scalar.add(out=qkT[s][:, r, icq * ICW:(icq + 1) * ICW],
                                          in_=ps[:], add=bqk_sb[:, r:r + 1])

                qkT_chunk(0)
                for icq in range(1, C_max):
                    with tc.If(R2v > 0):
                        qkT_chunk(icq)

                if parts == "qkt":
                    continue
                # v rows + ones col -> v_aug [P, NT, 4, 65]
                va = big.tile([P, NT, 4, D + 1], bf16, tag="vaug")
                nc.gpsimd.memset(va[:, :, :, D:D + 1], 1.0)
                for t in range(NT):
                    psv = psw.tile([P, C], f32, tag="w")
                    for c2 in range(2):
                        nc.tensor.matmul(psv[:], lhsT=hT[:, c2, t * P:(t + 1) * P],
                                         rhs=wv_sb[:, c2, :], start=(c2 == 0), stop=(c2 == 1))
                    for h in range(H):
                        nc.vector.tensor_copy(out=va[:, t, h, 0:D], in_=psv[:, h * D:(h + 1) * D])
                vaug[s] = va

                # uniform attention value u = (mean_v + bv) @ projT + pb
                psmv = psw.tile([1, H * (D + 1)], f32, tag="w")
                for t in range(NT):
                    nc.tensor.matmul(psmv[:], lhsT=ucol_sb[:], rhs=va[:, t, :, :],
                                     start=(t == 0), stop=(t == NT - 1))
                u_tmp = work.tile([1, C], f32, tag="utmp")
                for h in range(H):
                    nc.vector.tensor_copy(out=u_tmp[0:1, h * D:(h + 1) * D],
                                          in_=psmv[0:1, h * (D + 1):h * (D + 1) + D])
                nc.vector.tensor_tensor(out=u_tmp[:], in0=u_tmp[:], in1=bvrow_sb[:], op=Alu.add)
                mvT = work.tile([P, 2], bf16, tag="mvT")
                for c2 in range(2):
                    pst = psw.tile([P, 1], f32, tag="w")
                    nc.tensor.matmul(pst[:], lhsT=u_tmp[0:1, c2 * P:(c2 + 1) * P],
                                     rhs=ones1_sb[0:1, 0:1], start=True, stop=True)
                    nc.scalar.copy(out=mvT[:, c2:c2 + 1], in_=pst[:])
                psu = psw.tile([1, C], f32, tag="w")
                for c2 in range(2):
                    nc.tensor.matmul(psu[:], lhsT=mvT[:, c2:c2 + 1], rhs=proj_sb[:, c2, :],
                                     start=(c2 == 0), stop=(c2 == 1))
                u_row = work.tile([1, C], f32, tag="urow")
                nc.vector.tensor_tensor(out=u_row[:], in0=psu[:], in1=pbrow_sb[:], op=Alu.add)
                psub = psw.tile([P, C], f32, tag="w")
                nc.tensor.matmul(psub[:], lhsT=ones1_sb[:], rhs=u_row[:], start=True, stop=True)
                nc.scalar.copy(out=ub_sb[s][:], in_=psub[:])

                # dense pre-pass: x2 = x + u * (1 - m)   (in place on x_sb)
                for g in range(NT):
                    nc.vector.scalar_tensor_tensor(out=x_sb[s][:, g, :], in0=ub_sb[s][:],
                                                   scalar=minv[s][:, g:g + 1],
                                                   in1=x_sb[s][:, g, :],
                                                   op0=Alu.mult, op1=Alu.add)

                # ---- attention over valid region (per-sample dynamic tile
                # counts: Rv key-tiles; R2v gates the second query chunk) ----
                if parts == "vu":
                    continue
                for ic in range(C_max if parts != "mlp" else 0):
                    Rcond = Rv if ic == 0 else R2v
                    pT = pt2.tile([P, H, R_max, ICW], bf16, tag="pT", name="pT")
                    psav = [psacc.tile([D + 1, ICW], f32, tag=f"psav{h}", name=f"psav{h}")
                            for h in range(H)]
                    ktt_all = big1.tile([P, R_max, ICW], bf16, tag="ktt", name="ktt")

                    def prefetch(ic=ic, Rcond=Rcond, ktt_all=ktt_all):
                        for jt in range(R_max):
                            pair0 = (jt // 2) * 2
                            cond = None if (ic == 0 and pair0 == 0) else (Rcond > pair0)
                            nc.sync.dma_start(
                                out=ktt_all[:, jt, :],
                                in_=kt_ext[s, jt * P:(jt + 1) * P, ic * ICW:(ic + 1) * ICW],
                                cond=cond)

                    def jt_body(jt, ic=ic, pT=pT, psav=psav, ktt_all=ktt_all):
                        for h in range(H):
                            pss = psw.tile([P, ICW], f32, tag="w", name="pss")
                            nc.tensor.matmul(pss[:], lhsT=id_sb[:], rhs=ktt_all[:, jt, :],
                                             start=True, stop=False)
                            mo = (h % 2) * D
                            nc.tensor.matmul(pss[:],
                                             lhsT=qkT[s][mo:mo + D, 2 + h // 2, jt * P:(jt + 1) * P],
                                             rhs=qkT[s][mo:mo + D, h // 2, ic * ICW:(ic + 1) * ICW],
                                             start=False, stop=True)
                            nc.scalar.activation(out=pT[:, h, jt, :], in_=pss[:], func=Act.Exp,
                                                 bias=kb[s][:, jt:jt + 1], scale=1.0)
                        for h in range(H):
                            nc.tensor.matmul(psav[h][:], lhsT=va[:, jt, h, :],
                                             rhs=pT[:, h, jt, :],
                                             start=(jt == 0), stop=False,
                                             skip_group_check=True)

                    def chunk_tail(ic=ic, psav=psav):
                        # close the (possibly branch-shortened) accumulation
                        # groups with zero-contribution matmuls
                        for h in range(H):
                            nc.tensor.matmul(psav[h][:], lhsT=zcol_sb[:], rhs=zrow_sb[:],
                                             start=False, stop=True, skip_group_check=True)
                        # softmax denominators -> r = 1/s broadcast per head
                        for h in range(H):
                            if h % 2 == 0:
                                nc.vector.tensor_copy(out=stage_sb[32 * h:32 * h + 1, :],
                                                      in_=psav[h][D:D + 1, :])
                            else:
                                nc.scalar.copy(out=stage_sb[32 * h:32 * h + 1, :],
                                               in_=psav[h][D:D + 1, :])
                        r_sb = epi.tile([P, 2, ICW], f32, tag="rsb", name="rsb")
                        lntmp = epi.tile([P, ICW], f32, tag="lntmp", name="lntmp")
                        for c2 in range(2):
                            psr = psw.tile([P, ICW], f32, tag="w", name="psr")
                            nc.tensor.matmul(psr[:], lhsT=sel_sb[:, c2 * P:(c2 + 1) * P],
                                             rhs=stage_sb[:], start=True, stop=True)
                            nc.scalar.activation(out=lntmp[:], in_=psr[:], func=Act.Ln,
                                                 bias=0.0, scale=1.0)
                            nc.scalar.activation(out=r_sb[:, c2, :], in_=lntmp[:], func=Act.Exp,
                                                 bias=0.0, scale=-1.0)
                        oT = epi.tile([P, 2, ICW], bf16, tag="oT", name="oT")
                        for h in range(H):
                            mo = (h % 2) * D
                            nc.vector.tensor_tensor(out=oT[mo:mo + D, h // 2, :],
                                                    in0=psav[h][0:D, :],
                                                    in1=r_sb[mo:mo + D, h // 2, :], op=Alu.mult)
                        for c2 in range(2):
                            nc.vector.tensor_scalar(out=oT[:, c2, :], in0=oT[:, c2, :],
                                                    scalar1=bv_sb[:, c2:c2 + 1], scalar2=None,
                                                    op0=Alu.add)
                        for it in range(ICW // P):
                            g = ic * (ICW // P) + it
                            psp = psw.tile([P, C], f32, tag="w", name="psp")
                            for c2 in range(2):
                                nc.tensor.matmul(psp[:], lhsT=oT[:, c2, it * P:(it + 1) * P],
                                                 rhs=proj_sb[:, c2, :], start=(c2 == 0), stop=False)
                            nc.tensor.matmul(psp[:], lhsT=ones1_bf[:], rhs=pbrow_sb[:],
                                             start=False, stop=True)
                            nc.vector.scalar_tensor_tensor(out=x_sb[s][:, g, :], in0=psp[:],
                                                           scalar=mval[s][:, g:g + 1],
                                                           in1=x_sb[s][:, g, :],
                                                           op0=Alu.mult, op1=Alu.add)

                    def chunk(ic=ic, Rcond=Rcond):
                        for pr in range((R_max + 1) // 2):
                            jts = [jt for jt in (2 * pr, 2 * pr + 1) if jt < R_max]

                            def pair_body(jts=jts):
                                for jt in jts:
                                    jt_body(jt)

                            if ic == 0 and pr == 0:
                                pair_body()
                            else:
                                with tc.If(Rcond > 2 * pr):
                                    pair_body()
                        chunk_tail()

                    prefetch()
                    if ic == 0:
                        chunk()
                    else:
                        with tc.If(R2v > 0):
                            chunk()

            if parts in ("attn", "w", "ln", "qkt", "vu"):
                return_early = True
            else:
                return_early = False
            if parts in ("attn", "w", "ln", "qkt", "vu"):
                for s in range(SPC):
                    for t in range(NT):
                        nc.sync.dma_start(out=out_ext[s, t * P:(t + 1) * P, :],
                                          in_=x_sb[s][:, t, :])
            # =========== MLP phase: LN2 for both samples first (keeps the
            # exp/ln ACT table resident until all Exp work is done), then
            # fc1+gelu+fc2 per sample ===========
            h2T_keep = [None] * SPC
            for s in range(SPC if not return_early else 0):
                h2T = big.tile([P, 2, N], bf16, tag="h2T")
                layernorm_all(x_sb[s], s, 1, h2T)
                h2T_keep[s] = h2T
            for s in range(SPC if not return_early else 0):
                h2T = h2T_keep[s]
                mT = big1.tile([P, 8, N], bf16, tag="mT")
                for r in range(8):
                    for icol in range(2):
                        psf = psw.tile([P, ICW], f32, tag="w")
                        for c2 in range(2):
                            nc.tensor.matmul(psf[:], lhsT=fc1_sb[:, c2, r * P:(r + 1) * P],
                                             rhs=h2T[:, c2, icol * ICW:(icol + 1) * ICW],
                                             start=(c2 == 0), stop=(c2 == 1))
                        nc.scalar.activation(out=mT[:, r, icol * ICW:(icol + 1) * ICW],
                                             in_=psf[:], func=Act.Gelu,
                                             bias=bf1_sb[:, r:r + 1], scale=1.0)
                for t in range(NT):
                    psf2 = psw.tile([P, C], f32, tag="w")
                    for r in range(8):
                        nc.tensor.matmul(psf2[:], lhsT=mT[:, r, t * P:(t + 1) * P],
                                         rhs=fc2_sb[:, r, :], start=(r == 0), stop=False)
                    nc.tensor.matmul(psf2[:], lhsT=ones1_bf[:], rhs=f2b_sb[:],
                                     start=False, stop=True)
                    o_sb = outp.tile([P, C], f32, tag="o")
                    nc.vector.tensor_tensor(out=o_sb[:], in0=psf2[:], in1=x_sb[s][:, t, :],
                                            op=Alu.add)
                    nc.sync.dma_start(out=out_ext[s, t * P:(t + 1) * P, :], in_=o_sb[:])

    nc.finalize()
    return nc


def _prep(inputs):
    """Host-side preprocessing: sharding metadata + weight folding."""
    import ml_dtypes
    bf16 = ml_dtypes.bfloat16

    x = np.ascontiguousarray(np.asarray(inputs["x"], dtype=np.float32))
    K = np.asarray(inputs["K"], dtype=np.float32)
    n1 = np.asarray(inputs["n1"]).astype(np.int64)
    n2 = np.asarray(inputs["n2"]).astype(np.int64)
    nrc = n1 * n2
    scale = D ** -0.5

    g1 = np.asarray(inputs["ln1_g"], np.float32)
    b1 = np.asarray(inputs["ln1_b"], np.float32)
    g2 = np.asarray(inputs["ln2_g"], np.float32)
    b2 = np.asarray(inputs["ln2_b"], np.float32)
    qkv_w = np.asarray(inputs["qkv_w"], np.float32)
    qkv_b = np.asarray(inputs["qkv_b"], np.float32)

    Wqk = qkv_w[:2 * C]
    bqk = Wqk @ b1 + qkv_b[:2 * C]
    Wqk_eff = Wqk * g1[None, :]
    Wqk_eff = Wqk_eff.copy()
    Wqk_eff[:C] *= scale
    bqk = bqk.copy()
    bqk[:C] *= scale
    Wv = qkv_w[2 * C:]
    bv = Wv @ b1 + qkv_b[2 * C:]
    Wv_eff = Wv * g1[None, :]
    W1 = np.asarray(inputs["fc1_w"], np.float32)
    bf1 = W1 @ b2 + np.asarray(inputs["fc1_b"], np.float32)
    W1_eff = W1 * g2[None, :]

    sel4 = np.zeros((P, C), np.float32)
    for h in range(H):
        sel4[32 * h, h * D:(h + 1) * D] = 1.0

    iota_pt = (np.arange(P, dtype=np.float32)[:, None]
               + P * np.arange(NT, dtype=np.float32)[None, :])
    # f32 blob: order must match _build's _slicer table (nrcf appended per core)
    f32_parts = [
        np.ascontiguousarray(bqk.reshape(4, P).T),
        np.ascontiguousarray(bv.reshape(2, P).T),
        bv.reshape(1, C),
        np.ascontiguousarray(bf1.reshape(8, P).T),
        sel4,
        iota_pt,
    ]
    cf32_base = np.concatenate([p.ravel().astype(np.float32) for p in f32_parts])
    bf_parts = [
        np.ascontiguousarray(Wqk_eff.T).astype(bf16),
        np.ascontiguousarray(Wv_eff.T).astype(bf16),
        np.ascontiguousarray(np.asarray(inputs["proj_w"], np.float32).T).astype(bf16),
        np.asarray(inputs["proj_b"], np.float32).astype(bf16),
        np.ascontiguousarray(W1_eff.T).astype(bf16),
        np.ascontiguousarray(np.asarray(inputs["fc2_w"], np.float32).T).astype(bf16),
        np.asarray(inputs["fc2_b"], np.float32).astype(bf16),
        np.eye(P, dtype=np.float32).astype(bf16),
    ]
    cbf16 = np.concatenate([p.ravel() for p in bf_parts]).reshape(1, -1)
    shared = {"cbf16": cbf16}

    # balance: sort by nrc, pair largest with smallest
    order = np.argsort(nrc)
    pairs = [(int(order[B - 1 - i]), int(order[i])) for i in range(NCORES)]

    kt_all = np.ascontiguousarray(K.transpose(0, 2, 1)).astype(bf16)

    Rarr = ((nrc + P - 1) // P).astype(np.int32)
    Carr = ((nrc + ICW - 1) // ICW).astype(np.int32)
    in_maps = []
    for a, b in pairs:
        m = dict(shared)
        m["x"] = np.ascontiguousarray(x[[a, b]])
        m["kt"] = np.ascontiguousarray(kt_all[[a, b]])
        m["cf32"] = np.concatenate(
            [cf32_base, nrc[[a, b]].astype(np.float32)]).reshape(1, -1)
        fl = []
        for sidx in (a, b):
            R_i = int(Rarr[sidx])
            fl += [R_i, R_i if int(Carr[sidx]) >= 2 else 0]
        m["rflags"] = np.asarray(fl, np.int32).reshape(1, 2 * SPC)
        in_maps.append(m)

    R_max = int(np.max((nrc + P - 1) // P))
    C_max = int(np.max((nrc + ICW - 1) // ICW))
    return in_maps, pairs, R_max, C_max


def kernel(**inputs):
    from concourse.bass_utils import run_bass_kernel_spmd

    in_maps, pairs, R_max, C_max = _prep(inputs)
    nc = _build(R_max, C_max)
    res = run_bass_kernel_spmd(nc, in_maps, core_ids=list(range(NCORES)), trace=False)

    out = np.empty((B, N, C), np.float32)
    for c, (a, b) in enumerate(pairs):
        got = res.results[c]["out"]
        out[a] = got[0]
        out[b] = got[1]
    return out


if __name__ == "__main__":
    import reference as R

    inp = {k: np.asarray(v) for k, v in R.setup_inputs().items()}
    got = kernel(**inp)
    import jax
    exp = np.asarray(R.reference(**inp))
    rel = np.linalg.norm(got - exp) / np.linalg.norm(exp)
    print("Relative error:", rel)
